# revision 1
# baseline (speedup 1.0000x reference)
"""Trainium2 Bass kernel for nn_DirectedODRLoss (retrieval_knn).

Math (B=4096, D=256, k=25, scales (1,2,3)):
    dist²(i,j) = |f_i|² + |f_j|² − 2 f_i·f_j ;  y := −dist²  (computed directly
        by an augmented GEMM whose extra contraction rows carry −|f|²)
    topk:  per row, the 25 largest y  (= 25 smallest dist²) via DVE max8 +
        match_replace;  τ_i := 25th largest y;  σ_i = mean(sqrt(−y_clamped+eps))
    mutual knn mask:  y symmetric  ⇒  mutual(i,j) = [y_ij ≥ max(τ_i, τ_j)]
    Wn = exp(y·rσ_i·rσ_j − BIG·(1−mask·dir)),  dir = [lab_i ≤ lab_j]
    S_i = ΣWn + 1,  P = Wn/S + diag(1/S)   (diagonal added by DMA-accumulate)
    loss = (1/B)(C1 + C2/2 + C3/3) with
        C1 = <P, pen>, C2 = <P², pen>, C3 = <P³, pen> = <A, V>,
        A = P² (row shard), V = pen·Pᵀ (row shard), pen_ij = relu(s_i−s_j).

Sharding: rows are split across 8 cores. P is all-gathered (bf16) for the two
B³ GEMMs; Pᵀ is all-gathered from per-core PE-transposed shards. Final scalars
all-reduced. y/W strips are kept in fp16 in SBUF (halves SBUF + doubles DVE).
"""

import numpy as np

import concourse.bacc as bacc
import concourse.bass as bass
import concourse.mybir as mybir
import concourse.tile as tile
from concourse.masks import make_identity

F32 = mybir.dt.float32
F32R = mybir.dt.float32r
F16 = mybir.dt.float16
BF16 = mybir.dt.bfloat16
AX = mybir.AxisListType
OP = mybir.AluOpType
ACT = mybir.ActivationFunctionType

EPS = 1e-8
KNN = 25
BIG = 30000.0
NEG_INF = -1e30


def build_program(B=4096, D=256, NC=8):
    P = 128
    R = B // NC            # rows per core
    NMT = R // P           # row tiles per core
    TN = R                 # column tile (must equal R: Pᵀ AG block alignment)
    assert TN <= 512
    NNT = B // TN          # column tiles
    KC = B // P            # contraction chunks for the B-GEMMs
    GK = D // P            # contraction chunks for the Gram GEMM

    nc = bacc.Bacc("TRN2", target_bir_lowering=False, debug=False,
                   num_devices=NC)

    # ---- I/O ----------------------------------------------------------------
    ft2 = nc.dram_tensor("ft2", [D, R], F32, kind="ExternalInput")     # 2·F_shardᵀ
    ftf = nc.dram_tensor("ftf", [D, B], F32, kind="ExternalInput")     # Fᵀ (full)
    ff = nc.dram_tensor("ff", [B, D], F32, kind="ExternalInput")       # F (full)
    fsh = nc.dram_tensor("fsh", [R, D], F32, kind="ExternalInput")     # F shard
    srow = nc.dram_tensor("srow", [1, B], F32, kind="ExternalInput")   # scores
    smyrow = nc.dram_tensor("smyrow", [1, R], F32, kind="ExternalInput")
    scols = nc.dram_tensor("scols", [P, NMT], F32, kind="ExternalInput")
    lrow = nc.dram_tensor("lrow", [1, B], F32, kind="ExternalInput")
    lcols = nc.dram_tensor("lcols", [P, NMT], F32, kind="ExternalInput")
    loss_out = nc.dram_tensor("loss", [1, 1], F32, kind="ExternalOutput")

    # ---- internal DRAM ------------------------------------------------------
    pn_dram = nc.dram_tensor("pn_dram", [R, B], BF16)
    pt_dram = nc.dram_tensor("pt_dram", [B, R], BF16)                  # Pn_shardᵀ
    pfull = nc.dram_tensor("pfull", [NC * R, B], BF16, addr_space="Shared")
    ptfull = nc.dram_tensor("ptfull", [NC * B, R], BF16, addr_space="Shared")
    stats_in = nc.dram_tensor("stats_in", [1, 2 * R], F32)
    stats_out = nc.dram_tensor("stats_out", [NC, 2 * R], F32, addr_space="Shared")
    k2r_dram = nc.dram_tensor("k2r_dram", [2, B], F32)
    k2l_dram = nc.dram_tensor("k2l_dram", [2, R], F32)
    invs_dram = nc.dram_tensor("invs_dram", [P, NMT], F32)
    red_in = nc.dram_tensor("red_in", [1, 8], F32)
    red_out = nc.dram_tensor("red_out", [1, 8], F32, addr_space="Shared")

    rg = [list(range(NC))]

    with tile.TileContext(nc) as tc:
        with (
            tc.tile_pool(name="const", bufs=1) as constp,
            tc.tile_pool(name="io", bufs=3) as iop,
            tc.tile_pool(name="big", bufs=1) as bigp,
            tc.tile_pool(name="strip", bufs=3) as stripp,
            tc.tile_pool(name="cols", bufs=1) as colp,
            tc.tile_pool(name="work", bufs=2) as workp,
            tc.tile_pool(name="psum", bufs=1, space="PSUM") as psump,
        ):
            def ps_tile(tag, shape=None, dtype=F32):
                return psump.tile(shape or [P, TN], dtype, tag=tag, name=tag)

            # ============ stage A: squared norms ============================
            sqc = colp.tile([P, B // P], F32, tag="sqc")      # |f|² (full, cols)
            sqcs = colp.tile([P, NMT], F32, tag="sqcs")       # |f|² (shard, cols)
            for t in range(B // P):
                ftile = iop.tile([P, D], F32, tag="ftile")
                nc.sync.dma_start(ftile[:], ff[t * P:(t + 1) * P, :])
                scr = workp.tile([P, D], F32, tag="sqscr")
                nc.scalar.activation(scr[:], ftile[:], ACT.Square,
                                     accum_out=sqc[:, t:t + 1])
            for q in range(NMT):
                ftile = iop.tile([P, D], F32, tag="ftile")
                nc.sync.dma_start(ftile[:], fsh[q * P:(q + 1) * P, :])
                scr = workp.tile([P, D], F32, tag="sqscr")
                nc.scalar.activation(scr[:], ftile[:], ACT.Square,
                                     accum_out=sqcs[:, q:q + 1])
            # negate in column layout (round to f32r for the Gram matmul)
            sqcr = colp.tile([P, B // P], F32, tag="sqcr")
            sqcsr = colp.tile([P, NMT], F32, tag="sqcsr")
            nc.vector.tensor_scalar(sqcr[:], sqc[:], -1.0, None, OP.mult)
            nc.vector.tensor_scalar(sqcsr[:], sqcs[:], -1.0, None, OP.mult)

            # k2 (augmentation) operands, padded to 128 partitions.
            # lhsT rows: [1, −|f_m|², 0...];  rhs rows: [−|f_n|², 1, 0...]
            # (partition-offset-1 SBUF writes are not allowed, so assemble the
            #  two rows in DRAM and load them with one base-0 DMA)
            ones_row = constp.tile([1, R], F32, tag="ones_row")
            nc.vector.memset(ones_row[:], 1.0)
            for t in range(B // R):
                nc.sync.dma_start(k2r_dram[1:2, t * R:(t + 1) * R], ones_row[:])
            nc.sync.dma_start(k2l_dram[0:1, :], ones_row[:])
            # row layouts: flat[g] with g = c*128 + p  ⇐  sbuf cols [p, c]
            nc.sync.dma_start(bass.AP(k2r_dram, 0, [[1, P], [P, B // P]]), sqcr[:])
            nc.sync.dma_start(bass.AP(k2l_dram, R, [[1, P], [P, NMT]]), sqcsr[:])
            lhs_k2 = constp.tile([P, R], F32, tag="lhs_k2")
            rhs_k2 = constp.tile([P, TN], F32, tag="rhs_k2")
            nc.vector.memset(lhs_k2[:], 0.0)
            nc.vector.memset(rhs_k2[:], 0.0)
            nc.sync.dma_start(lhs_k2[0:2, :], k2l_dram[:, :])

            # ============ stage A2: Gram → y (fp16 strips) ==================
            ft2_sb = constp.tile([P, GK * R], F32, tag="ft2_sb")
            for g in range(GK):
                nc.sync.dma_start(ft2_sb[:, g * R:(g + 1) * R],
                                  ft2[g * P:(g + 1) * P, :])

            y_all = bigp.tile([P, NMT * B], F32, tag="ybuf")
            for nt in range(NNT):
                gps = [ps_tile(f"pa{m}") for m in range(NMT)]
                for g in range(GK):
                    gt = iop.tile([P, TN], F32, tag="rt", name="gt")
                    nc.sync.dma_start(gt[:], ftf[g * P:(g + 1) * P,
                                                 nt * TN:(nt + 1) * TN])
                    for mt in range(NMT):
                        nc.tensor.matmul(
                            gps[mt][:],
                            ft2_sb[:, g * R + mt * P: g * R + (mt + 1) * P],
                            gt[:],
                            start=(g == 0), stop=False)
                nc.sync.dma_start(rhs_k2[0:2, :],
                                  k2r_dram[:, nt * TN:(nt + 1) * TN])
                for mt in range(NMT):
                    nc.tensor.matmul(
                        gps[mt][:],
                        lhs_k2[:, mt * P:(mt + 1) * P],
                        rhs_k2[:],
                        start=False, stop=True)
                    nc.scalar.activation(
                        y_all[:, mt * B + nt * TN: mt * B + (nt + 1) * TN],
                        gps[mt][:], ACT.Copy)

            # ============ stage B: top-k → τ, rσ ============================
            vals = colp.tile([P, NMT * 32], F32, tag="vals")
            yt_cols = colp.tile([P, NMT], F32, tag="yt_cols")
            rs_cols = colp.tile([P, NMT], F32, tag="rs_cols")
            ssum = colp.tile([P, NMT], F32, tag="ssum")
            eps_c = constp.tile([P, 1], F32, tag="eps_c")
            nc.vector.memset(eps_c[:], EPS)
            for mt in range(NMT):
                ys = y_all[:, mt * B:(mt + 1) * B]
                sa = stripp.tile([P, B], F32, tag="strip", name="sa")
                sb = stripp.tile([P, B], F32, tag="strip", name="sb")
                nc.scalar.activation(sa[:], ys, ACT.Copy)
                v = vals[:, mt * 32:(mt + 1) * 32]
                nc.vector.max(out=v[:, 0:8], in_=sa[:])
                nc.vector.match_replace(out=sb[:], in_to_replace=v[:, 0:8],
                                        in_values=sa[:], imm_value=NEG_INF)
                nc.vector.max(out=v[:, 8:16], in_=sb[:])
                nc.vector.match_replace(out=sa[:], in_to_replace=v[:, 8:16],
                                        in_values=sb[:], imm_value=NEG_INF)
                nc.vector.max(out=v[:, 16:24], in_=sa[:])
                nc.vector.match_replace(out=sb[:], in_to_replace=v[:, 16:24],
                                        in_values=sa[:], imm_value=NEG_INF)
                nc.vector.max(out=v[:, 24:32], in_=sb[:])
                # τ_i = 25th largest y
                nc.vector.tensor_copy(yt_cols[:, mt:mt + 1], v[:, 24:25])
                # σ_i = mean sqrt(max(d,0)+eps) over 25 NN;  d = −y
                c25 = workp.tile([P, KNN], F32, tag="c25")
                nc.vector.tensor_scalar(c25[:], v[:, 0:KNN], 0.0, None, OP.min)
                s25 = workp.tile([P, KNN], F32, tag="s25")
                nc.scalar.activation(s25[:], c25[:], ACT.Sqrt,
                                     bias=eps_c[:, 0:1], scale=-1.0,
                                     accum_out=ssum[:, mt:mt + 1])
            nc.vector.reciprocal(rs_cols[:], ssum[:])
            nc.vector.tensor_scalar(rs_cols[:], rs_cols[:], float(KNN), None,
                                    OP.mult)

            # stats all-gather: flat per-rank [τ(R) ++ rσ(R)], both in
            # shard-row order g_local = c*128 + p  →  AG output is directly
            # the full vector in global row order.
            nc.sync.dma_start(bass.AP(stats_in, 0, [[1, P], [P, NMT]]),
                              yt_cols[:])
            nc.sync.dma_start(bass.AP(stats_in, R, [[1, P], [P, NMT]]),
                              rs_cols[:])
            nc.gpsimd.collective_compute(
                "AllGather", OP.bypass, replica_groups=rg,
                ins=[stats_in.ap().opt()], outs=[stats_out.ap().opt()])

            def stat_bcast_ap(off):
                return bass.AP(stats_out, off, [[0, P], [2 * R, NC], [1, R]])

            yt_b = stripp.tile([P, B], F32, tag="strip", name="yt_b")
            rs_b = stripp.tile([P, B], F32, tag="strip", name="rs_b")
            lab_b = stripp.tile([P, B], F32, tag="strip", name="lab_b")
            nc.sync.dma_start(yt_b[:].rearrange("a (r q) -> a r q", r=NC),
                              stat_bcast_ap(0))
            nc.sync.dma_start(rs_b[:].rearrange("a (r q) -> a r q", r=NC),
                              stat_bcast_ap(R))
            nc.sync.dma_start(lab_b[:], bass.AP(lrow, 0, [[0, P], [1, B]]))

            lab_c = colp.tile([P, NMT], F32, tag="lab_c")
            s_c = colp.tile([P, NMT], F32, tag="s_c")
            nc.sync.dma_start(lab_c[:], lcols[:, :])
            nc.sync.dma_start(s_c[:], scols[:, :])

            # ============ stage W: Wn, S, Pn, C1 ============================
            srcols = colp.tile([P, NMT * NNT], F32, tag="srcols")
            c1cols = colp.tile([P, NMT * NNT], F32, tag="c1cols")
            s_b = constp.tile([P, B], F32, tag="s_b")
            nc.sync.dma_start(s_b[:], bass.AP(srow, 0, [[0, P], [1, B]]))

            for mt in range(NMT):
                for nt in range(NNT):
                    ys = y_all[:, mt * B + nt * TN: mt * B + (nt + 1) * TN]
                    thr = workp.tile([P, TN], F32, tag="w1", name="thr")
                    nc.vector.tensor_scalar(thr[:], yt_b[:, nt * TN:(nt + 1) * TN],
                                            yt_cols[:, mt:mt + 1], None, OP.max)
                    keep = workp.tile([P, TN], F32, tag="w2", name="keep")
                    nc.vector.tensor_tensor(keep[:], ys, thr[:], OP.is_ge)
                    dirk = workp.tile([P, TN], F32, tag="w3", name="dirk")
                    nc.vector.tensor_scalar(dirk[:], lab_b[:, nt * TN:(nt + 1) * TN],
                                            lab_c[:, mt:mt + 1], None, OP.is_ge)
                    mask = workp.tile([P, TN], F32, tag="w4", name="mask")
                    nc.vector.tensor_tensor(mask[:], keep[:], dirk[:], OP.mult)
                    # 1 → 0,  0 → −BIG
                    nc.vector.tensor_scalar(mask[:], mask[:], BIG, -BIG,
                                            OP.mult, op1=OP.add)
                    e = workp.tile([P, TN], F32, tag="w1", name="e")
                    nc.vector.tensor_tensor(e[:], ys, rs_b[:, nt * TN:(nt + 1) * TN],
                                            OP.mult)
                    nc.vector.tensor_tensor(e[:], e[:], mask[:], OP.add)
                    # Wn = exp(e·rσ_i), in place over y
                    nc.scalar.activation(ys, e[:], ACT.Exp,
                                         scale=rs_cols[:, mt:mt + 1],
                                         accum_out=srcols[:, mt * NNT + nt:
                                                          mt * NNT + nt + 1])
                    # C1 partial: Σ Wn·pen (row scale by 1/S applied later)
                    pen = workp.tile([P, TN], F32, tag="w2", name="pen")
                    nc.scalar.activation(pen[:], s_b[:, nt * TN:(nt + 1) * TN],
                                         ACT.Relu, bias=s_c[:, mt:mt + 1],
                                         scale=-1.0)
                    prod = workp.tile([P, TN], F32, tag="w3", name="prod")
                    nc.gpsimd.tensor_tensor(prod[:], ys, pen[:], OP.mult)
                    junk = workp.tile([P, TN], F32, tag="w1", name="junk")
                    nc.scalar.activation(junk[:], prod[:], ACT.Copy,
                                         accum_out=c1cols[:, mt * NNT + nt:
                                                          mt * NNT + nt + 1])

            # S = ΣWn + 1 ;  invS = 1/S
            invS = colp.tile([P, NMT], F32, tag="invS")
            Scol = colp.tile([P, NMT], F32, tag="Scol")
            for mt in range(NMT):
                nc.vector.reduce_sum(Scol[:, mt:mt + 1],
                                     srcols[:, mt * NNT:(mt + 1) * NNT], axis=AX.X)
            nc.vector.tensor_scalar(Scol[:], Scol[:], 1.0, None, OP.add)
            nc.vector.reciprocal(invS[:], Scol[:])
            nc.sync.dma_start(invs_dram[:, :], invS[:])

            # Pn tiles (bf16) → DRAM
            for mt in range(NMT):
                for nt in range(NNT):
                    pn_t = workp.tile([P, TN], BF16, tag="pn_t")
                    nc.vector.tensor_scalar(
                        pn_t[:], y_all[:, mt * B + nt * TN: mt * B + (nt + 1) * TN],
                        invS[:, mt:mt + 1], None, OP.mult)
                    nc.sync.dma_start(pn_dram[mt * P:(mt + 1) * P,
                                              nt * TN:(nt + 1) * TN], pn_t[:])

            # C1 finalize (per-row 1/S)
            c1v = colp.tile([P, 1], F32, tag="c1v")
            c1r = colp.tile([P, NMT], F32, tag="c1r")
            for mt in range(NMT):
                nc.vector.reduce_sum(c1r[:, mt:mt + 1],
                                     c1cols[:, mt * NNT:(mt + 1) * NNT], axis=AX.X)
            nc.vector.tensor_tensor(c1r[:], c1r[:], invS[:], OP.mult)
            nc.vector.reduce_sum(c1v[:], c1r[:], axis=AX.X)

            # ============ diagonal fix: P += diag(1/S) ======================
            invs_rowf = colp.tile([1, R], F32, tag="invs_rowf")
            nc.sync.dma_start(invs_rowf[:].rearrange("a (c p) -> a c p", p=P),
                              bass.AP(invs_dram, 0, [[0, 1], [1, NMT], [NMT, P]]))
            invs_row = colp.tile([1, R], BF16, tag="invs_row")
            nc.vector.tensor_copy(invs_row[:], invs_rowf[:])
            rank = nc.gpsimd.partition_id()
            diag_ap = pn_dram.ap().rearrange("a b -> () (a b)")[
                0:1, bass.ds(rank * R, R, B + 1)]
            nc.gpsimd.dma_start(diag_ap, invs_row[0:1, :], accum_op=OP.add)

            # ============ transposes → lhsT (and Pᵀ AG input) ===============
            ident = constp.tile([P, P], BF16, tag="ident")
            make_identity(nc, ident[:])
            lp_buf = bigp.tile([P, 2 * KC * R], BF16, tag="ybuf", name="lp_buf")
            lhsT = lp_buf[:, 0:KC * R]
            for q in range(NMT):
                for kb in range(KC):
                    src = workp.tile([P, P], BF16, tag="tsrc")
                    nc.sync.dma_start(src[:], pn_dram[q * P:(q + 1) * P,
                                                      kb * P:(kb + 1) * P])
                    pst = ps_tile(f"pv{kb % 4}", shape=[P, P], dtype=BF16)
                    nc.tensor.transpose(pst[:], src[:], ident[:])
                    nc.any.tensor_copy(
                        lhsT[:, kb * R + q * P: kb * R + (q + 1) * P], pst[:])
            # write Pnᵀ shard for the Pᵀ all-gather
            for kb in range(KC):
                nc.sync.dma_start(pt_dram[kb * P:(kb + 1) * P, :],
                                  lhsT[:, kb * R:(kb + 1) * R])

            # ============ all-gathers ======================================
            nc.gpsimd.collective_compute(
                "AllGather", OP.bypass, replica_groups=rg,
                ins=[pn_dram.ap().opt()], outs=[pfull.ap().opt()])
            nc.gpsimd.collective_compute(
                "AllGather", OP.bypass, replica_groups=rg,
                ins=[pt_dram.ap().opt()], outs=[ptfull.ap().opt()])

            # ============ penᵀ chunks (lhsT of the V-GEMM) ==================
            smy_b = constp.tile([P, R], F32, tag="smy_b")
            nc.sync.dma_start(smy_b[:], bass.AP(smyrow, 0, [[0, P], [1, R]]))
            sfc = colp.tile([P, KC], F32, tag="sfc")       # −s_j, col layout
            nc.sync.dma_start(sfc[:], bass.AP(srow, 0, [[1, P], [P, KC]]))
            nc.vector.tensor_scalar(sfc[:], sfc[:], -1.0, None, OP.mult)
            penT = lp_buf[:, KC * R:2 * KC * R]
            for kb in range(KC):
                nc.scalar.activation(penT[:, kb * R:(kb + 1) * R], smy_b[:],
                                     ACT.Relu, bias=sfc[:, kb:kb + 1], scale=1.0)

            # ============ main GEMMs + contractions =========================
            c2cols = colp.tile([P, NMT * NNT], F32, tag="c2cols")
            c3cols = colp.tile([P, NMT * NNT], F32, tag="c3cols")
            for nt in range(NNT):
                pa = [ps_tile(f"pa{m}") for m in range(NMT)]
                pv = [ps_tile(f"pv{m}") for m in range(NMT)]
                for kb in range(KC):
                    rt = iop.tile([P, TN], BF16, tag="rt", name="rt")
                    nc.sync.dma_start(rt[:], pfull[kb * P:(kb + 1) * P,
                                                   nt * TN:(nt + 1) * TN])
                    for m in range(NMT):
                        nc.tensor.matmul(pa[m][:],
                                         lhsT[:, kb * R + m * P: kb * R + (m + 1) * P],
                                         rt[:], start=(kb == 0), stop=(kb == KC - 1))
                    rtv = iop.tile([P, TN], BF16, tag="rtv", name="rtv")
                    nc.sync.dma_start(rtv[:], ptfull[nt * B + kb * P:
                                                     nt * B + (kb + 1) * P, :])
                    for m in range(NMT):
                        nc.tensor.matmul(pv[m][:],
                                         penT[:, kb * R + m * P: kb * R + (m + 1) * P],
                                         rtv[:], start=(kb == 0), stop=(kb == KC - 1))
                for m in range(NMT):
                    zs = workp.tile([P, TN], F32, tag="w4", name="zs")
                    nc.scalar.activation(zs[:], pv[m][:], ACT.Copy)
                    pen = workp.tile([P, TN], F32, tag="w2", name="pen")
                    nc.scalar.activation(pen[:], s_b[:, nt * TN:(nt + 1) * TN],
                                         ACT.Relu, bias=s_c[:, m:m + 1], scale=-1.0)
                    prodA = workp.tile([P, TN], F32, tag="w3", name="prodA")
                    nc.vector.tensor_tensor(prodA[:], pa[m][:], pen[:], OP.mult)
                    junk = workp.tile([P, TN], F32, tag="w1", name="junk")
                    nc.scalar.activation(junk[:], prodA[:], ACT.Copy,
                                         accum_out=c2cols[:, nt * NMT + m:
                                                          nt * NMT + m + 1])
                    prodZ = workp.tile([P, TN], F32, tag="w3", name="prodZ")
                    nc.vector.tensor_tensor(prodZ[:], pa[m][:], zs[:], OP.mult)
                    junk2 = workp.tile([P, TN], F32, tag="w1", name="junk2")
                    nc.scalar.activation(junk2[:], prodZ[:], ACT.Copy,
                                         accum_out=c3cols[:, nt * NMT + m:
                                                          nt * NMT + m + 1])

            # ============ final reduction ==================================
            c2v = colp.tile([P, 1], F32, tag="c2v")
            c3v = colp.tile([P, 1], F32, tag="c3v")
            nc.vector.reduce_sum(c2v[:], c2cols[:], axis=AX.X)
            nc.vector.reduce_sum(c3v[:], c3cols[:], axis=AX.X)
            tot = colp.tile([P, 1], F32, tag="tot")
            nc.vector.tensor_scalar(tot[:], c2v[:], 0.5, None, OP.mult)
            nc.vector.tensor_tensor(tot[:], tot[:], c1v[:], OP.add)
            nc.vector.tensor_scalar(c3v[:], c3v[:], 1.0 / 3.0, None, OP.mult)
            nc.vector.tensor_tensor(tot[:], tot[:], c3v[:], OP.add)

            ones_c = constp.tile([P, 1], F32, tag="ones_c")
            nc.vector.memset(ones_c[:], 1.0)
            fin = ps_tile("pa0", shape=[1, 8])
            nc.tensor.matmul(fin[:, 0:1], tot[:], ones_c[:], start=True, stop=True)
            lsb = colp.tile([1, 8], F32, tag="lsb")
            nc.vector.memset(lsb[:], 0.0)
            nc.scalar.activation(lsb[:, 0:1], fin[:, 0:1], ACT.Copy,
                                 scale=1.0 / float(B))
            nc.sync.dma_start(red_in[:, :], lsb[:])
            nc.gpsimd.collective_compute(
                "AllReduce", OP.add, replica_groups=rg,
                ins=[red_in.ap().opt()], outs=[red_out.ap().opt()])
            nc.sync.dma_start(loss_out[:, :], red_out[0:1, 0:1])

    nc.compile()
    return nc


def make_inputs(features, scores, labels, B, D, NC):
    """Build the per-core input maps from full inputs."""
    R = B // NC
    P = 128
    NMT = R // P
    f = np.ascontiguousarray(features, dtype=np.float32)
    s = np.ascontiguousarray(scores, dtype=np.float32).reshape(B)
    lab = np.asarray(labels).astype(np.float32).reshape(B)
    ftf = np.ascontiguousarray(f.T)
    in_maps = []
    for c in range(NC):
        sh = slice(c * R, (c + 1) * R)
        in_maps.append({
            "ft2": np.ascontiguousarray(2.0 * f[sh].T),
            "ftf": ftf,
            "ff": f,
            "fsh": np.ascontiguousarray(f[sh]),
            "srow": s.reshape(1, B),
            "smyrow": np.ascontiguousarray(s[sh]).reshape(1, R),
            "scols": np.ascontiguousarray(s[sh].reshape(NMT, P).T),
            "lrow": lab.reshape(1, B),
            "lcols": np.ascontiguousarray(lab[sh].reshape(NMT, P).T),
        })
    return in_maps


_cached = {}


def kernel(features, scores, labels):
    B, D = features.shape
    NC = 8
    key = (B, D)
    if key not in _cached:
        _cached[key] = build_program(B=B, D=D, NC=NC)
    nc = _cached[key]
    from concourse.bass_utils import run_bass_kernel_spmd
    in_maps = make_inputs(features, scores, labels, B, D, NC)
    res = run_bass_kernel_spmd(nc, in_maps, core_ids=list(range(NC)))
    out = res.results[0]["loss"]
    return np.float32(out.reshape(())[()])



# revision 8
# speedup vs baseline: 1.5881x; 1.5881x over previous
"""Trainium2 Bass kernel for nn_DirectedODRLoss (retrieval_knn).

Math (B=4096, D=256, k=25, scales (1,2,3)):
    dist²(i,j) = |f_i|² + |f_j|² − 2 f_i·f_j ;  y := −dist²  (computed directly
        by an augmented GEMM whose extra contraction rows carry −|f|²)
    topk:  per row, 25 largest y (= 25 smallest dist²).  Done hierarchically:
        per 512-col block top-32 via DVE max8 + match_replace (pipelined under
        the Gram GEMM), then a 256-candidate merge per row strip.
        τ_i := 25th largest y;  σ_i = mean(sqrt(−y+eps)) over the 25.
    mutual knn mask:  y symmetric  ⇒  mutual(i,j) = [y_ij ≥ max(τ_i, τ_j)]
    Wn = exp((y·rσ_j − BIG·(1−mask·dir))·rσ_i),  dir = [lab_i ≤ lab_j]
    S_i = ΣWn + 1,  P = (Wn + diag(1))·(1/S)  (diag via one-hot dsel input)
    loss = (1/B)(C1 + C2/2 + C3/3) with
        C1 = <P, pen>  (fused into stage W),  pen_ij = relu(s_i−s_j)
        A = P_sh·P  (row-parallel fp8 GEMM over all-gathered P), C2 = <A, pen>
        T = A·P    (second fp8 GEMM, same rhs),               C3 = <T, pen>

Precision: Gram in f32r (full PE rate at 512-wide tiles), y/W strips fp16
(2× DVE), P and A quantized to fp8e4 for the two B³ GEMMs with DoubleRow
perf mode (2× PE).  P is all-gathered once in fp8 (16 MB);  there is no
Pᵀ all-gather — C3 uses T = A·P with lhsT built by on-chip PE transposes.
pn_dram uses a tiled layout [nt][p][mt][c] so GEMM rhs DMA descriptors are
1 KiB contiguous.
"""

import numpy as np

import concourse.bacc as bacc
import concourse.bass as bass
import concourse.mybir as mybir
import concourse.tile as tile
from concourse.masks import make_identity

F32 = mybir.dt.float32
F32R = mybir.dt.float32r
F16 = mybir.dt.float16
F8 = mybir.dt.float8e4
AX = mybir.AxisListType
OP = mybir.AluOpType
ACT = mybir.ActivationFunctionType
PM = mybir.MatmulPerfMode

EPS = 1e-8
KNN = 25
BIG = 30000.0
SENT = -60000.0          # fp16-safe "removed" sentinel for topk scratch


def build_program(B=4096, D=256, NC=8):
    P = 128
    R = B // NC            # rows per core (512)
    NMT = R // P           # row tiles per core (4)
    TN = 512               # column tile
    NNT = B // TN          # column tiles (8)
    KC = B // P            # 128-row contraction chunks (32)
    KB2 = KC // 2          # 256-row DoubleRow chunks (16)
    GK = D // P            # contraction chunks for the Gram GEMM (2)
    ROWB = NMT * TN        # bytes per partition-row in a pn_dram nt-block (2048)

    nc = bacc.Bacc("TRN2", target_bir_lowering=False, debug=False,
                   num_devices=NC)

    # ---- I/O ----------------------------------------------------------------
    ft2 = nc.dram_tensor("ft2", [D, R], F32R, kind="ExternalInput")     # 2·F_shardᵀ
    ftf = nc.dram_tensor("ftf", [D, B], F32R, kind="ExternalInput")     # Fᵀ (full)
    ff = nc.dram_tensor("ff", [B, D], F32, kind="ExternalInput")        # F (full)
    fsh = nc.dram_tensor("fsh", [R, D], F32, kind="ExternalInput")      # F shard
    srow = nc.dram_tensor("srow", [1, B], F32, kind="ExternalInput")    # scores
    scols = nc.dram_tensor("scols", [P, NMT], F32, kind="ExternalInput")
    lrow = nc.dram_tensor("lrow", [1, B], F32, kind="ExternalInput")
    lcols = nc.dram_tensor("lcols", [P, NMT], F32, kind="ExternalInput")
    dsel = nc.dram_tensor("dsel", [1, NNT], F32, kind="ExternalInput")  # one-hot rank
    loss_out = nc.dram_tensor("loss", [1, 1], F32, kind="ExternalOutput")

    # ---- internal DRAM ------------------------------------------------------
    # pn_dram tiled layout: elem(nt, p, mt, c) at nt·(P·ROWB) + p·ROWB + mt·TN + c
    pn_dram = nc.dram_tensor("pn_dram", [1, R * B], F8)
    pfull = nc.dram_tensor("pfull", [1, NC * R * B], F8, addr_space="Shared")
    nsq_dram = nc.dram_tensor("nsq_dram", [1, B], F32)
    stats_in = nc.dram_tensor("stats_in", [1, 2 * R], F32)
    stats_out = nc.dram_tensor("stats_out", [NC, 2 * R], F32, addr_space="Shared")
    red_in = nc.dram_tensor("red_in", [1, 8], F32)
    red_out = nc.dram_tensor("red_out", [1, 8], F32, addr_space="Shared")

    rg = [list(range(NC))]

    with tile.TileContext(nc) as tc:
        with (
            tc.tile_pool(name="const", bufs=1) as constp,
            tc.tile_pool(name="io", bufs=3) as iop,
            tc.tile_pool(name="big", bufs=1) as bigp,
            tc.tile_pool(name="cols", bufs=1) as colp,
            tc.tile_pool(name="work", bufs=2) as workp,
            tc.tile_pool(name="psum", bufs=1, space="PSUM") as psump,
        ):
            def ps_tile(tag, shape=None, dtype=F32):
                return psump.tile(shape or [P, TN], dtype, tag=tag, name=tag)

            # ============ constants / broadcast strips ======================
            ident16 = constp.tile([P, P], F16, tag="ident16")
            make_identity(nc, ident16[:])
            ident8 = constp.tile([P, P], F8, tag="ident8")
            make_identity(nc, ident8[:])
            eps_c = constp.tile([P, 1], F32, tag="eps_c")
            nc.vector.memset(eps_c[:], EPS)

            s_b32 = constp.tile([P, B], F32, tag="s_b32")
            nc.sync.dma_start(s_b32[:], bass.AP(srow, 0, [[0, P], [1, B]]))
            lab_tmp = workp.tile([P, B], F32, tag="btmp", name="lab_tmp")
            nc.sync.dma_start(lab_tmp[:], bass.AP(lrow, 0, [[0, P], [1, B]]))
            lab_b16 = constp.tile([P, B], F16, tag="lab_b16")
            nc.vector.tensor_copy(lab_b16[:], lab_tmp[:])
            lab_c = colp.tile([P, NMT], F32, tag="lab_c")
            s_c = colp.tile([P, NMT], F32, tag="s_c")
            nc.sync.dma_start(lab_c[:], lcols[:, :])
            nc.sync.dma_start(s_c[:], scols[:, :])
            dselc = colp.tile([P, NNT], F32, tag="dselc")
            nc.sync.dma_start(dselc[:], bass.AP(dsel, 0, [[0, P], [1, NNT]]))

            # ============ stage A: squared norms ============================
            sqc = colp.tile([P, KC], F32, tag="sqc")          # |f|² full, cols
            sqcs = colp.tile([P, NMT], F32, tag="sqcs")       # |f|² shard, cols
            for t in range(KC):
                ftile = iop.tile([P, D], F32, tag="ftile")
                nc.sync.dma_start(ftile[:], ff[t * P:(t + 1) * P, :])
                scr = workp.tile([P, D], F32, tag="sqscr")
                nc.scalar.activation(scr[:], ftile[:], ACT.Square,
                                     accum_out=sqc[:, t:t + 1])
            for q in range(NMT):
                ftile = iop.tile([P, D], F32, tag="ftile")
                nc.sync.dma_start(ftile[:], fsh[q * P:(q + 1) * P, :])
                scr = workp.tile([P, D], F32, tag="sqscr")
                nc.scalar.activation(scr[:], ftile[:], ACT.Square,
                                     accum_out=sqcs[:, q:q + 1])
            sqcr = colp.tile([P, KC], F32, tag="sqcr")
            sqcsr = colp.tile([P, NMT], F32, tag="sqcsr")
            nc.vector.tensor_scalar(sqcr[:], sqc[:], -1.0, None, OP.mult)
            nc.vector.tensor_scalar(sqcsr[:], sqcs[:], -1.0, None, OP.mult)

            # −|f_j|² broadcast strip (row layout flat g = c*128 + p)
            nc.sync.dma_start(bass.AP(nsq_dram, 0, [[1, P], [P, KC]]), sqcr[:])
            nsq_tmp = workp.tile([P, B], F32, tag="btmp", name="nsq_tmp")
            nc.sync.dma_start(nsq_tmp[:], bass.AP(nsq_dram, 0, [[0, P], [1, B]]))
            nsq_b16 = constp.tile([P, B], F16, tag="nsq_b16")
            nc.vector.tensor_copy(nsq_b16[:], nsq_tmp[:])

            ft2_sb = constp.tile([P, GK * R], F32R, tag="ft2_sb")
            for g in range(GK):
                nc.sync.dma_start(ft2_sb[:, g * R:(g + 1) * R],
                                  ft2[g * P:(g + 1) * P, :])

            # ============ stage A2+B: Gram → y (fp16) + per-block top32 =====
            y16 = bigp.tile([P, NMT * B], F16, tag="y16")
            vals = colp.tile([P, NMT * NNT * 32], F16, tag="vals")
            for nt in range(NNT):
                gps = [ps_tile(f"pa{m}") for m in range(NMT)]
                for g in range(GK):
                    gt = iop.tile([P, TN], F32R, tag="gt", name="gt")
                    nc.sync.dma_start(gt[:], ftf[g * P:(g + 1) * P,
                                                 nt * TN:(nt + 1) * TN])
                    for mt in range(NMT):
                        nc.tensor.matmul(
                            gps[mt][:],
                            ft2_sb[:, g * R + mt * P: g * R + (mt + 1) * P],
                            gt[:],
                            start=(g == 0), stop=(g == GK - 1))
                for mt in range(NMT):
                    ys = y16[:, mt * B + nt * TN: mt * B + (nt + 1) * TN]
                    # y = (2F_sh·Fᵀ)(PSUM) − |f_i|² (bias) − |f_j|² (strip add)
                    nc.scalar.activation(ys, gps[mt][:], ACT.Identity,
                                         bias=sqcsr[:, mt:mt + 1])
                    nc.vector.tensor_tensor(ys, ys,
                                            nsq_b16[:, nt * TN:(nt + 1) * TN],
                                            OP.add)
                    # block top-32 (descending) of this [128, 512] tile
                    v = vals[:, (mt * NNT + nt) * 32:(mt * NNT + nt + 1) * 32]
                    t0 = workp.tile([P, TN], F16, tag="tks", name="t0")
                    t1 = workp.tile([P, TN], F16, tag="tks2", name="t1")
                    nc.vector.max(out=v[:, 0:8], in_=ys)
                    nc.vector.match_replace(out=t0[:], in_to_replace=v[:, 0:8],
                                            in_values=ys, imm_value=SENT)
                    nc.vector.max(out=v[:, 8:16], in_=t0[:])
                    nc.vector.match_replace(out=t1[:], in_to_replace=v[:, 8:16],
                                            in_values=t0[:], imm_value=SENT)
                    nc.vector.max(out=v[:, 16:24], in_=t1[:])
                    nc.vector.match_replace(out=t0[:], in_to_replace=v[:, 16:24],
                                            in_values=t1[:], imm_value=SENT)
                    nc.vector.max(out=v[:, 24:32], in_=t0[:])

            # ============ stage B2: merge block candidates → τ, rσ ==========
            yt_cols = colp.tile([P, NMT], F32, tag="yt_cols")
            rs_cols = colp.tile([P, NMT], F32, tag="rs_cols")
            ssum = colp.tile([P, NMT], F32, tag="ssum")
            for mt in range(NMT):
                cand = vals[:, mt * NNT * 32:(mt + 1) * NNT * 32]
                w = workp.tile([P, 32], F16, tag="wtop", name="w")
                m0 = workp.tile([P, NNT * 32], F16, tag="mscr", name="m0")
                m1 = workp.tile([P, NNT * 32], F16, tag="mscr2", name="m1")
                nc.vector.max(out=w[:, 0:8], in_=cand)
                nc.vector.match_replace(out=m0[:], in_to_replace=w[:, 0:8],
                                        in_values=cand, imm_value=SENT)
                nc.vector.max(out=w[:, 8:16], in_=m0[:])
                nc.vector.match_replace(out=m1[:], in_to_replace=w[:, 8:16],
                                        in_values=m0[:], imm_value=SENT)
                nc.vector.max(out=w[:, 16:24], in_=m1[:])
                nc.vector.match_replace(out=m0[:], in_to_replace=w[:, 16:24],
                                        in_values=m1[:], imm_value=SENT)
                nc.vector.max(out=w[:, 24:32], in_=m0[:])
                # τ_i = 25th largest y
                nc.vector.tensor_copy(yt_cols[:, mt:mt + 1], w[:, 24:25])
                # σ_i = mean sqrt(max(d,0)+eps) over 25 NN;  d = −y
                c25 = workp.tile([P, KNN], F16, tag="c25")
                nc.vector.tensor_scalar(c25[:], w[:, 0:KNN], 0.0, None, OP.min)
                s25 = workp.tile([P, KNN], F32, tag="s25")
                nc.scalar.activation(s25[:], c25[:], ACT.Sqrt,
                                     bias=eps_c[:, 0:1], scale=-1.0,
                                     accum_out=ssum[:, mt:mt + 1])
            nc.vector.reciprocal(rs_cols[:], ssum[:])
            nc.vector.tensor_scalar(rs_cols[:], rs_cols[:], float(KNN), None,
                                    OP.mult)

            # stats all-gather: flat per-rank [τ(R) ++ rσ(R)], shard-row order
            nc.sync.dma_start(bass.AP(stats_in, 0, [[1, P], [P, NMT]]),
                              yt_cols[:])
            nc.sync.dma_start(bass.AP(stats_in, R, [[1, P], [P, NMT]]),
                              rs_cols[:])
            nc.gpsimd.collective_compute(
                "AllGather", OP.bypass, replica_groups=rg,
                ins=[stats_in.ap().opt()], outs=[stats_out.ap().opt()])

            def stat_bcast_ap(off):
                return bass.AP(stats_out, off, [[0, P], [2 * R, NC], [1, R]])

            yt_tmp = workp.tile([P, B], F32, tag="btmp", name="yt_tmp")
            nc.sync.dma_start(yt_tmp[:].rearrange("a (r q) -> a r q", r=NC),
                              stat_bcast_ap(0))
            yt_b16 = constp.tile([P, B], F16, tag="yt_b16")
            nc.vector.tensor_copy(yt_b16[:], yt_tmp[:])
            rs_tmp = workp.tile([P, B], F32, tag="btmp", name="rs_tmp")
            nc.sync.dma_start(rs_tmp[:].rearrange("a (r q) -> a r q", r=NC),
                              stat_bcast_ap(R))
            rs_b16 = constp.tile([P, B], F16, tag="rs_b16")
            nc.vector.tensor_copy(rs_b16[:], rs_tmp[:])

            # ============ stage W: Wn, S, C1, Pn(fp8), Pᵀ (mt-pipelined) ====
            srcols = colp.tile([P, NMT * NNT], F32, tag="srcols")
            c1cols = colp.tile([P, NMT * NNT], F32, tag="c1cols")
            invS = colp.tile([P, NMT], F32, tag="invS")
            Scol = colp.tile([P, NMT], F32, tag="Scol")
            lhsT_P = bigp.tile([P, KC * R], F8, tag="lhsT_P")
            lhsT_A = bigp.tile([P, KC * R], F8, tag="lhsT_A")
            lp_view = lhsT_P[:].rearrange("p (kb m) -> p kb m", kb=KC)
            la_view = lhsT_A[:].rearrange("p (kb m) -> p kb m", kb=KC)

            for mt in range(NMT):
                for nt in range(NNT):
                    ys = y16[:, mt * B + nt * TN: mt * B + (nt + 1) * TN]
                    # (lab_j < lab_i) · BIG
                    dthr = workp.tile([P, TN], F16, tag="w1", name="dthr")
                    nc.vector.tensor_scalar(
                        dthr[:], lab_b16[:, nt * TN:(nt + 1) * TN],
                        lab_c[:, mt:mt + 1], BIG, OP.is_lt, op1=OP.mult)
                    thr = workp.tile([P, TN], F16, tag="w2", name="thr")
                    nc.vector.tensor_scalar(thr[:],
                                            yt_b16[:, nt * TN:(nt + 1) * TN],
                                            yt_cols[:, mt:mt + 1], None, OP.max)
                    nc.vector.tensor_tensor(thr[:], thr[:], dthr[:], OP.add)
                    keep = workp.tile([P, TN], F16, tag="w3", name="keep")
                    nc.vector.tensor_tensor(keep[:], ys, thr[:], OP.is_ge)
                    # 1 → 0,  0 → −BIG
                    nc.vector.tensor_scalar(keep[:], keep[:], BIG, -BIG,
                                            OP.mult, op1=OP.add)
                    e = workp.tile([P, TN], F16, tag="w4", name="e")
                    nc.vector.tensor_tensor(e[:], ys,
                                            rs_b16[:, nt * TN:(nt + 1) * TN],
                                            OP.mult)
                    nc.vector.tensor_tensor(e[:], e[:], keep[:], OP.add)
                    # Wn = exp(e·rσ_i), in place over y16
                    nc.scalar.activation(ys, e[:], ACT.Exp,
                                         scale=rs_cols[:, mt:mt + 1],
                                         accum_out=srcols[:, mt * NNT + nt:
                                                          mt * NNT + nt + 1])
                    # diagonal: += ident·dsel[nt] on the (mt) sub-block
                    dg = workp.tile([P, P], F16, tag="wdg", name="dg")
                    nc.vector.tensor_scalar(dg[:], ident16[:],
                                            dselc[:, nt:nt + 1], None, OP.mult)
                    ysd = y16[:, mt * B + nt * TN + mt * P:
                              mt * B + nt * TN + (mt + 1) * P]
                    nc.vector.tensor_tensor(ysd, ysd, dg[:], OP.add)
                    # C1 partial: Σ Wn·pen (row scale 1/S applied later;
                    # diag contributes 0 since pen_ii = 0)
                    pen = workp.tile([P, TN], F16, tag="wpen", name="pen")
                    nc.scalar.activation(pen[:], s_b32[:, nt * TN:(nt + 1) * TN],
                                         ACT.Relu, bias=s_c[:, mt:mt + 1],
                                         scale=-1.0)
                    prod = workp.tile([P, TN], F16, tag="wprod", name="prod")
                    nc.gpsimd.tensor_tensor(prod[:], ys, pen[:], OP.mult)
                    junk = workp.tile([P, TN], F16, tag="wjunk", name="junk")
                    nc.scalar.activation(junk[:], prod[:], ACT.Copy,
                                         accum_out=c1cols[:, mt * NNT + nt:
                                                          mt * NNT + nt + 1])
                # S = ΣWn + 1 ;  invS = 1/S   (this mt only)
                nc.vector.reduce_sum(Scol[:, mt:mt + 1],
                                     srcols[:, mt * NNT:(mt + 1) * NNT], axis=AX.X)
                nc.vector.tensor_scalar(Scol[:, mt:mt + 1], Scol[:, mt:mt + 1],
                                        1.0, None, OP.add)
                nc.vector.reciprocal(invS[:, mt:mt + 1], Scol[:, mt:mt + 1])
                # Pn tiles: fp16 P → fp8 DMA out + fp16 PE transposes → fp8 lhsT_P
                for nt in range(NNT):
                    pn16 = workp.tile([P, TN], F16, tag="pn16", name="pn16")
                    nc.vector.tensor_scalar(
                        pn16[:], y16[:, mt * B + nt * TN: mt * B + (nt + 1) * TN],
                        invS[:, mt:mt + 1], None, OP.mult)
                    pn8 = workp.tile([P, TN], F8, tag="pn8", name="pn8")
                    nc.vector.tensor_copy(pn8[:], pn16[:])
                    nc.sync.dma_start(
                        bass.AP(pn_dram, nt * (P * ROWB) + mt * TN,
                                [[ROWB, P], [1, TN]]),
                        pn8[:])
                    pst = ps_tile("pt", shape=[P, TN], dtype=F16)
                    for b in range(NMT):
                        nc.tensor.transpose(pst[:, b * P:(b + 1) * P],
                                            pn16[:, b * P:(b + 1) * P],
                                            ident16[:])
                    nc.any.tensor_copy(
                        lp_view[:, nt * NMT:(nt + 1) * NMT,
                                mt * P:(mt + 1) * P],
                        pst[:].rearrange("p (b m) -> p b m", b=NMT))

            # ============ all-gather P (fp8, 16 MB) =========================
            nc.gpsimd.collective_compute(
                "AllGather", OP.bypass, replica_groups=rg,
                ins=[pn_dram.ap().opt()], outs=[pfull.ap().opt()])

            # ============ A = P_sh·P  (fp8 DoubleRow), C2, Aᵀ ==============
            c2cols = colp.tile([P, NMT * NNT], F32, tag="c2cols")
            c3cols = colp.tile([P, NMT * NNT], F32, tag="c3cols")

            def rhs_ap(KB, nt):
                return bass.AP(pfull,
                               (KB // 2) * (R * B) + nt * (P * ROWB)
                               + (KB % 2) * (2 * TN),
                               [[ROWB, P], [1, 2 * TN]])

            csum2 = colp.tile([P, NMT * NNT], F32, tag="csum2")
            csum3 = colp.tile([P, NMT * NNT], F32, tag="csum3")

            def gemm_pass(lview, ccols, csums, do_transpose):
                for nt in range(NNT):
                    pa = [ps_tile(f"pa{m}") for m in range(NMT)]
                    for KB in range(KB2):
                        rt = iop.tile([P, 2 * TN], F8, tag="rt", name="rt")
                        nc.sync.dma_start(rt[:], rhs_ap(KB, nt))
                        rv = rt[:].rearrange("p (i n) -> p i n", i=2)
                        for m in range(NMT):
                            nc.tensor.matmul(
                                pa[m][:],
                                lview[:, 2 * KB:2 * KB + 2, m * P:(m + 1) * P],
                                rv,
                                start=(KB == 0), stop=(KB == KB2 - 1),
                                perf_mode=PM.DoubleRow)
                    for m in range(NMT):
                        pen = workp.tile([P, TN], F16, tag="wpen", name="gpen")
                        nc.scalar.activation(pen[:],
                                             s_b32[:, nt * TN:(nt + 1) * TN],
                                             ACT.Relu, bias=s_c[:, m:m + 1],
                                             scale=-1.0)
                        prodc = workp.tile([P, TN], F32, tag="wprodc",
                                           name="prodc")
                        nc.vector.tensor_tensor(prodc[:], pa[m][:], pen[:],
                                                OP.mult)
                        junk = workp.tile([P, TN], F16, tag="wjunk",
                                          name="gjunk")
                        nc.scalar.activation(junk[:], prodc[:], ACT.Copy,
                                             accum_out=ccols[:, nt * NMT + m:
                                                             nt * NMT + m + 1])
                        # row sums of A/T (for fp8-bias-cancelling renorm);
                        # for the A pass this op doubles as the fp16 cast
                        # feeding the Aᵀ transposes
                        a16 = workp.tile([P, TN], F16, tag="a16", name="a16")
                        nc.scalar.activation(a16[:], pa[m][:], ACT.Copy,
                                             accum_out=csums[:, nt * NMT + m:
                                                             nt * NMT + m + 1])
                        if do_transpose:
                            pst = ps_tile("pt", shape=[P, TN], dtype=F16)
                            for b in range(NMT):
                                nc.tensor.transpose(pst[:, b * P:(b + 1) * P],
                                                    a16[:, b * P:(b + 1) * P],
                                                    ident16[:])
                            nc.any.tensor_copy(
                                la_view[:, nt * NMT:(nt + 1) * NMT,
                                        m * P:(m + 1) * P],
                                pst[:].rearrange("p (b m) -> p b m", b=NMT))

            gemm_pass(lp_view, c2cols, csum2, do_transpose=True)
            # ============ T = A·P  (fp8 DoubleRow), C3 ======================
            gemm_pass(la_view, c3cols, csum3, do_transpose=False)

            # ============ final reduction ==================================
            c1r = colp.tile([P, NMT], F32, tag="c1r")
            for mt in range(NMT):
                nc.vector.reduce_sum(c1r[:, mt:mt + 1],
                                     c1cols[:, mt * NNT:(mt + 1) * NNT], axis=AX.X)
            nc.vector.tensor_tensor(c1r[:], c1r[:], invS[:], OP.mult)
            c1v = colp.tile([P, 1], F32, tag="c1v")
            nc.vector.reduce_sum(c1v[:], c1r[:], axis=AX.X)
            # C2_i = Σ_k A_ik pen_ik / Σ_k A_ik  (A is exactly row-stochastic
            # without fp8 rounding, so dividing by the measured row sum
            # cancels the systematic fp8 quantization bias); same for C3/T.
            c2r = colp.tile([P, NMT], F32, tag="c2r")
            c3r = colp.tile([P, NMT], F32, tag="c3r")
            s2r = colp.tile([P, NMT], F32, tag="s2r")
            s3r = colp.tile([P, NMT], F32, tag="s3r")
            cc2 = c2cols[:].rearrange("p (nt m) -> p m nt", m=NMT)
            cc3 = c3cols[:].rearrange("p (nt m) -> p m nt", m=NMT)
            cs2 = csum2[:].rearrange("p (nt m) -> p m nt", m=NMT)
            cs3 = csum3[:].rearrange("p (nt m) -> p m nt", m=NMT)
            for m in range(NMT):
                nc.vector.reduce_sum(c2r[:, m:m + 1], cc2[:, m, :], axis=AX.X)
                nc.vector.reduce_sum(c3r[:, m:m + 1], cc3[:, m, :], axis=AX.X)
                nc.vector.reduce_sum(s2r[:, m:m + 1], cs2[:, m, :], axis=AX.X)
                nc.vector.reduce_sum(s3r[:, m:m + 1], cs3[:, m, :], axis=AX.X)
            nc.vector.reciprocal(s2r[:], s2r[:])
            nc.vector.reciprocal(s3r[:], s3r[:])
            nc.vector.tensor_tensor(c2r[:], c2r[:], s2r[:], OP.mult)
            nc.vector.tensor_tensor(c3r[:], c3r[:], s3r[:], OP.mult)
            c2v = colp.tile([P, 1], F32, tag="c2v")
            c3v = colp.tile([P, 1], F32, tag="c3v")
            nc.vector.reduce_sum(c2v[:], c2r[:], axis=AX.X)
            nc.vector.reduce_sum(c3v[:], c3r[:], axis=AX.X)
            tot = colp.tile([P, 1], F32, tag="tot")
            nc.vector.tensor_scalar(tot[:], c2v[:], 0.5, None, OP.mult)
            nc.vector.tensor_tensor(tot[:], tot[:], c1v[:], OP.add)
            nc.vector.tensor_scalar(c3v[:], c3v[:], 1.0 / 3.0, None, OP.mult)
            nc.vector.tensor_tensor(tot[:], tot[:], c3v[:], OP.add)

            ones_c = constp.tile([P, 1], F32, tag="ones_c")
            nc.vector.memset(ones_c[:], 1.0)
            fin = ps_tile("pfin", shape=[1, 8])
            nc.tensor.matmul(fin[:, 0:1], tot[:], ones_c[:], start=True,
                             stop=True)
            lsb = colp.tile([1, 8], F32, tag="lsb")
            nc.vector.memset(lsb[:], 0.0)
            nc.scalar.activation(lsb[:, 0:1], fin[:, 0:1], ACT.Copy,
                                 scale=1.0 / float(B))
            nc.sync.dma_start(red_in[:, :], lsb[:])
            nc.gpsimd.collective_compute(
                "AllReduce", OP.add, replica_groups=rg,
                ins=[red_in.ap().opt()], outs=[red_out.ap().opt()])
            nc.sync.dma_start(loss_out[:, :], red_out[0:1, 0:1])

    nc.compile()
    return nc


def make_inputs(features, scores, labels, B, D, NC):
    """Build the per-core input maps from full inputs."""
    R = B // NC
    P = 128
    NMT = R // P
    NNT = B // 512
    f = np.ascontiguousarray(features, dtype=np.float32)
    s = np.ascontiguousarray(scores, dtype=np.float32).reshape(B)
    lab = np.asarray(labels).astype(np.float32).reshape(B)
    ftf = np.ascontiguousarray(f.T)
    in_maps = []
    for c in range(NC):
        sh = slice(c * R, (c + 1) * R)
        dsel = np.zeros((1, NNT), dtype=np.float32)
        dsel[0, c] = 1.0
        in_maps.append({
            "ft2": np.ascontiguousarray(2.0 * f[sh].T),
            "ftf": ftf,
            "ff": f,
            "fsh": np.ascontiguousarray(f[sh]),
            "srow": s.reshape(1, B),
            "scols": np.ascontiguousarray(s[sh].reshape(NMT, P).T),
            "lrow": lab.reshape(1, B),
            "lcols": np.ascontiguousarray(lab[sh].reshape(NMT, P).T),
            "dsel": dsel,
        })
    return in_maps


_cached = {}


def kernel(features, scores, labels):
    B, D = features.shape
    NC = 8
    key = (B, D)
    if key not in _cached:
        _cached[key] = build_program(B=B, D=D, NC=NC)
    nc = _cached[key]
    from concourse.bass_utils import run_bass_kernel_spmd
    in_maps = make_inputs(features, scores, labels, B, D, NC)
    res = run_bass_kernel_spmd(nc, in_maps, core_ids=list(range(NC)))
    out = res.results[0]["loss"]
    return np.float32(out.reshape(())[()])
